# revision 5
# baseline (speedup 1.0000x reference)
"""CAM (channel attention) module kernel for Trainium2, 8-core data-parallel.

Reference computation (per batch b, channel c):
    v = x[b,c]                         # (P=3, HW=4096)
    energy = v @ v.T                   # (3,3) Gram matrix
    en = rowmax(energy) - energy
    att = softmax(en, axis=-1)
    out = att @ v                      # (3, 4096)
    y[b,c] = gamma * out + x[b,c]

Sharding: batch dim (B=8) across the 8 NeuronCores; no cross-core comms.

Design (v2): the kernel computes in bf16 end-to-end (tolerance is 2e-2;
bf16 round-trip error is ~1.7e-3). The host converts x to bf16 inside
kernel(); the device program reads/writes bf16 HBM tensors, halving the
25.2 MB/core fp32 HBM traffic to 12.6 MB (DMA floor ~35 us/core measured
at ~358 GB/s/core), which turns a memory-bound problem into a
compute-bound one.

Engine assignment is driven by per-op microbenchmarks on this HW
([128,4096] ops, steady state):
  - DVE scalar_tensor_tensor never engages 2x perf modes (~4.2-4.8 us,
    any dtype) -> used only where its fused mult+reduce saves a pass
    (Gram cross terms with accum_out).
  - DVE tensor_scalar bf16 hits 4x (1.17 us) -> phase-2 scale products.
  - DVE tensor_tensor bf16 hits 2x (2.37 us) -> phase-2 adds, packed as
    [128, 3*4096] ops across the 3 output rows to amortize overhead.
  - ACT big ops cost ~3.2-4.0 us (SBUF-src errata); ACT takes the Gram
    diagonal (Square + accum) plus `k_act` of the 9 scale products to
    balance the two engines (~31 us/group each, 2 groups/core).

Per 128-channel group (channels on partitions):
  load vb [128,3,4096] bf16 (1 DMA, 24KB/partition contiguous)
  diag:  ACT Square(vb_i) accum -> E[ii]
  cross: DVE STT(vb_i * vb_j) accum -> E[ij], mirrored on ACT
  softmax smalls (fp32, [128,<=9]) -> Cf = gamma*att + I
  phase2: a_k = Cf[k,1]*vb1, b_k = Cf[k,0]*vb0, c_k = Cf[k,2]*vb2
          (9 products, k_act of them on ACT, rest DVE TS 4x)
          a += b ; y = a + c  (two packed [128,3*4096] TT ops)
  store y [128,3,4096] bf16 (1 DMA)
"""

import numpy as np
import ml_dtypes

import concourse.bacc as bacc
import concourse.mybir as mybir
import concourse.tile as tile
from concourse.bass_utils import run_bass_kernel_spmd

B, C, P, H, W = 8, 256, 3, 64, 64
HW = H * W
N_CORES = 8
PARTS = 128

F32 = mybir.dt.float32
BF16 = mybir.dt.bfloat16
Alu = mybir.AluOpType
Act = mybir.ActivationFunctionType
BF = ml_dtypes.bfloat16


def build_nc(C_=C, HW_=HW, repeat=1, k_act=3, packed=True, store_eng="gpsimd"):
    """Build the per-core Bass program. Each core sees x:(C_,P,HW_) bf16,
    gamma:(1,1) f32, and writes y:(C_,P,HW_) bf16.

    repeat>1 re-runs the whole computation (same I/O, idempotent) that many
    times in one program — used by test.py to time the kernel by slope.
    k_act: how many of the 9 phase-2 scale products per group run on ACT
    (the rest run on DVE tensor_scalar at 4x). packed: fuse the phase-2
    adds across the 3 output rows into [128, 3*HW] ops.
    """
    assert C_ % PARTS == 0
    n_groups = C_ // PARTS

    nc = bacc.Bacc("TRN2", target_bir_lowering=False, debug=False)

    x_d = nc.dram_tensor("x", [C_, P, HW_], BF16, kind="ExternalInput")
    g_d = nc.dram_tensor("gamma", [1, 1], F32, kind="ExternalInput")
    y_d = nc.dram_tensor("y", [C_, P, HW_], BF16, kind="ExternalOutput")

    with tile.TileContext(nc) as tc:
        with (
            tc.tile_pool(name="consts", bufs=1) as consts,
            tc.tile_pool(name="vbpool", bufs=2) as vbpool,
            tc.tile_pool(name="scratch", bufs=1) as scratch,
            tc.tile_pool(name="work", bufs=1) as work,
            tc.tile_pool(name="ypool", bufs=1) as ypool,
            tc.tile_pool(name="smalls", bufs=2) as smalls,
        ):
            # --- constants (once) ---
            gsb = consts.tile([1, 1], F32)
            nc.sync.dma_start(gsb[:], g_d[:])
            gamma_bc = consts.tile([PARTS, 1], F32)
            nc.gpsimd.partition_broadcast(gamma_bc[:], gsb[:])

            ident = consts.tile([PARTS, 9], F32)
            nc.vector.memset(ident[:], 0.0)
            for i in range(P):
                nc.vector.memset(ident[:, 4 * i : 4 * i + 1], 1.0)

            # --- software-pipelined emission -------------------------------
            # Iteration it: phase-2 of group `prev` (its Cf is ready from the
            # previous iteration) is emitted BEFORE phase-1/softmax of the
            # current group, so neither in-order engine queue head-blocks:
            #   ACT queue: products(prev) | diag(cur) | mirrors/exp(cur)
            #   DVE queue: TS-products(prev) | cross(cur) | TT+store(prev)
            #              | min/sum/recip/A/Cf(cur)
            def emit_products(st):
                """9 scale products for group st: k_act on ACT, rest DVE TS."""
                vb, Cf = st["vb"], st["Cf"]
                a = work.tile([PARTS, P, HW_], BF16, tag="a", bufs=1)
                b = work.tile([PARTS, P, HW_], BF16, tag="b", bufs=1)
                c = work.tile([PARTS, P, HW_], BF16, tag="c", bufs=1)
                n_act = 0
                for dst, j in ((a, 1), (b, 0), (c, 2)):
                    for row in range(P):
                        sc = Cf[:, 3 * row + j : 3 * row + j + 1]
                        if n_act < k_act:
                            nc.scalar.activation(dst[:, row, :], vb[:, j, :],
                                                 Act.Copy, scale=sc)
                        else:
                            nc.vector.tensor_scalar(dst[:, row, :], vb[:, j, :],
                                                    sc, None, op0=Alu.mult)
                        n_act += 1
                st["abc"] = (a, b, c)

            def emit_phase1(g):
                """Load + Gram diag (ACT) + cross (DVE) for group g."""
                cs = slice(g * PARTS, (g + 1) * PARTS)
                vb = vbpool.tile([PARTS, P, HW_], BF16, tag="vb", bufs=2)
                nc.sync.dma_start(vb[:], x_d[cs, :, :])
                E = smalls.tile([PARTS, 9], F32)
                for i in range(P):
                    scr = scratch.tile([PARTS, HW_], BF16, tag="scr_act", bufs=1)
                    nc.scalar.activation(
                        scr[:], vb[:, i, :], Act.Square,
                        accum_out=E[:, 4 * i : 4 * i + 1],
                    )
                for i, j, col in ((0, 1, 1), (1, 2, 5), (0, 2, 2)):
                    scr = scratch.tile([PARTS, HW_], BF16, tag="scr_dve", bufs=1)
                    nc.vector.scalar_tensor_tensor(
                        scr[:], vb[:, i, :], 1.0, vb[:, j, :],
                        op0=Alu.bypass, op1=Alu.mult,
                        accum_out=E[:, col : col + 1],
                    )
                return {"g": g, "cs": cs, "vb": vb, "E": E}

            def emit_tt_store(st):
                """Packed adds + store for group st (reads st['abc']).
                Store is dispatched from the idle GPSIMD queue so the SP
                FIFO carries only loads: a store waiting on TT2 would
                otherwise head-block the next group's load dispatch."""
                a, b, c = st["abc"]
                yt = ypool.tile([PARTS, P, HW_], BF16, tag="y", bufs=1)
                nc.vector.tensor_tensor(a[:], a[:], b[:], op=Alu.add)
                nc.vector.tensor_tensor(yt[:], a[:], c[:], op=Alu.add)
                eng = nc.gpsimd if store_eng == "gpsimd" else nc.sync
                eng.dma_start(y_d[st["cs"], :, :], yt[:])

            def emit_softmax(st):
                """Mirrors + softmax smalls -> st['Cf'] = gamma*att + I."""
                E = st["E"]
                for src, dst in ((1, 3), (5, 7), (2, 6)):
                    nc.scalar.copy(E[:, dst : dst + 1], E[:, src : src + 1])
                E3 = E.rearrange("p (i j) -> p i j", j=P)
                M = smalls.tile([PARTS, P, 1], F32)
                # reference computes softmax(rowmax - E); softmax is shift
                # invariant, so use (rowmin - E): exponents stay <= 0.
                nc.vector.tensor_reduce(M[:], E3, axis=mybir.AxisListType.X,
                                        op=Alu.min)
                EX = smalls.tile([PARTS, P, P], F32)
                for i in range(P):
                    nc.scalar.activation(
                        EX[:, i, :], E3[:, i, :], Act.Exp,
                        scale=-1.0, bias=M[:, i, 0:1],
                    )
                S = smalls.tile([PARTS, P, 1], F32)
                nc.vector.tensor_reduce(S[:], EX[:], axis=mybir.AxisListType.X,
                                        op=Alu.add)
                R = smalls.tile([PARTS, P, 1], F32)
                nc.vector.reciprocal(R[:], S[:])
                A = smalls.tile([PARTS, P, P], F32)
                nc.vector.tensor_mul(A[:], EX[:], R[:].broadcast_to([PARTS, P, P]))
                Cf = smalls.tile([PARTS, 9], F32)
                nc.vector.scalar_tensor_tensor(
                    Cf[:].rearrange("p (i j) -> p i j", j=P), A[:],
                    gamma_bc[:, 0:1],
                    ident[:].rearrange("p (i j) -> p i j", j=P),
                    op0=Alu.mult, op1=Alu.add,
                )
                st["Cf"] = Cf

            n_iters = n_groups * repeat
            prev = None
            for it in range(n_iters):
                if prev is not None:
                    emit_products(prev)
                cur = emit_phase1(it % n_groups)
                if prev is not None:
                    emit_tt_store(prev)
                emit_softmax(cur)
                prev = cur
            emit_products(prev)
            emit_tt_store(prev)

    nc.compile()
    return nc


_NC_CACHE = {}


def _get_nc(C_=C, HW_=HW):
    key = (C_, HW_)
    if key not in _NC_CACHE:
        _NC_CACHE[key] = build_nc(C_, HW_)
    return _NC_CACHE[key]


def run_full(x: np.ndarray, gamma: np.ndarray, **runner_kwargs):
    """Run on all 8 cores; returns the raw BassKernelResults."""
    x = np.asarray(x, dtype=np.float32)
    gamma = np.asarray(gamma, dtype=np.float32)
    assert x.shape == (B, C, P, H, W), x.shape

    xb = x.astype(BF)
    nc = _get_nc()
    in_maps = [
        {
            "x": np.ascontiguousarray(xb[k]).reshape(C, P, HW),
            "gamma": gamma.reshape(1, 1),
        }
        for k in range(N_CORES)
    ]
    return run_bass_kernel_spmd(
        nc, in_maps, core_ids=list(range(N_CORES)), **runner_kwargs
    )


def kernel(x: np.ndarray, gamma: np.ndarray) -> np.ndarray:
    res = run_full(x, gamma)
    y = np.stack([np.asarray(res.results[k]["y"]) for k in range(N_CORES)])
    return y.astype(np.float32).reshape(B, C, P, H, W)
